# revision 50
# baseline (speedup 1.0000x reference)
"""Sliding-window gated attention on 8 TRN2 NeuronCores.

Sharding: data/sequence parallel, no collectives. 2 batches x 4096 tokens
= 8192 tokens -> 8 shards of 1024 owned tokens (core c: batch c//4,
segment c%4). Each shard also receives a 256-token halo of x on the left
(the sliding window W=256 only ever reaches one block back), so every
core computes its outputs fully locally. For segment-0 cores the halo is
dummy data that the attention mask zeroes out.

Per-core layout is feature-major ("transposed"): xT [1024 dim, 1280 tok].
  rs      = 1/||x_t||           (ones-vector matmul over squared chunks)
  xhatT   = xT * rs             (row-broadcast via gpsimd partition_broadcast)
  qT,kT   = W^T @ xhatT         [feat, tok]   (bf16 matmuls)
  v       = xhatT^T @ Wv        [tok, feat]   (+ interleaved ones columns)
  scoresT = kT_h^T @ qT_h       [kpos, q]  per (head, 128-token key chunk)
  eT      = exp(scoresT) * mask{0,1}       (no max subtraction; scores are O(1))
  AV      = [v_h | 1]^T @ eT    [65, 1024]: rows 0-63 unnormalized out,
                                row 64 = softmax denominator (2-bank PSUM)
  attgT   = AV[0:64] * (sigmoid(gate)/denom)  broadcast along partitions
  yT      = W_out^T @ attgT     [dim, tok]  (bf16 output)
RMS-norm gamma*sqrt(1024), the 1/sqrt(64) attention scale, and gamma for
the gate projection are folded into the weights host-side; weights and
xhat are bf16 (halves weight DMA, full PE rate).

Schedule: the Q/K passes for feature tiles 2-7 and the second half of the
V pass are interleaved into the attention head loop as PE "filler" so the
ACT/DVE-bound attention phase overlaps the projections, and const-fed
dummy matmuls pad the remaining PE idle windows -- both keep the PE HAM
activity monitor from re-throttling the clock to 1.2GHz.  Attention
epilogue per head: one [65,1024] AV accumulation, denominator row
evacuated by ACT, reciprocal+gate multiply on DVE, gpsimd partition
broadcast, one [64,1024] DVE multiply.
"""
import numpy as np
import ml_dtypes

import concourse.bass as bass
import concourse.tile as tile
from concourse import bacc, mybir
from concourse.bass_utils import run_bass_kernel_spmd

F32 = mybir.dt.float32
F32R = mybir.dt.float32r
BF16 = mybir.dt.bfloat16
AF = mybir.ActivationFunctionType

P = 128
DIM = 1024
HEADS = 16
DH = 64
WIN = 256
OWN = 1024          # owned tokens per core
HALO = 256
SL = OWN + HALO     # local tokens (1280)
KK = DIM // P       # 8 contraction chunks
FT = HEADS // 2     # 8 feature tiles (2 heads each)
TCH = SL // P       # 10 local token chunks
NB = OWN // WIN     # 4 owned blocks
NCORES = 8

# q-span (in owned-token coords) of each global key chunk g, and width
_G_SPAN = [(0, 256), (0, 256), (0, 512), (0, 512), (256, 512), (256, 512),
           (512, 512), (512, 512), (768, 256), (768, 256)]
# column offset of chunk g's mask inside the [128, 2048] mask tensor
_G_MASK = [1024, 1280, 0, 0, 0, 0, 0, 0, 1536, 1792]
for _g in (3, 5, 7):
    _G_MASK[_g] = 512
# statically-valid column range of each g's eT tile (outside: mask is 0,
# so exp is skipped there and the mask multiply writes the zeros)
_G_VALID = [(0, 128), (0, 256), (0, 384), (128, 384), (0, 384), (128, 384),
            (0, 384), (128, 384), (0, 256), (128, 128)]


def _round_f32r(a):
    u = np.ascontiguousarray(a, dtype=np.float32).view(np.uint32)
    r = ((u.astype(np.uint64) + 0x800) & 0xFFFFF000).astype(np.uint32)
    return r.view(np.float32).reshape(a.shape)


def _band(c):
    """{0,1} validity for key-chunk-position kp vs in-block query ql."""
    kp = np.arange(P)[:, None]
    ql = np.arange(WIN)[None, :]
    diff = 256 + ql - 128 * c - kp
    return ((diff >= 0) & (diff <= WIN)).astype(np.float32)


def _masks(first_segment):
    m_even = np.concatenate([_band(2), _band(0)], axis=1)
    m_odd = np.concatenate([_band(3), _band(1)], axis=1)
    zeros = np.zeros_like(_band(0))
    g0 = zeros if first_segment else _band(0)
    g1 = zeros if first_segment else _band(1)
    m = np.concatenate([m_even, m_odd, g0, g1, _band(2), _band(3)], axis=1)
    return m.astype(ml_dtypes.bfloat16)


def build():
    nc = bacc.Bacc("TRN2", target_bir_lowering=False, debug=False,
                   num_devices=NCORES)
    xT_d = nc.dram_tensor("xT", [DIM, SL], F32, kind="ExternalInput")
    wq_d = nc.dram_tensor("Wq", [DIM, DIM], BF16, kind="ExternalInput")
    wk_d = nc.dram_tensor("Wk", [DIM, DIM], BF16, kind="ExternalInput")
    wv_d = nc.dram_tensor("Wv", [DIM, DIM], BF16, kind="ExternalInput")
    wg_d = nc.dram_tensor("Wg", [DIM, HEADS], BF16, kind="ExternalInput")
    bg_d = nc.dram_tensor("bg", [HEADS], F32, kind="ExternalInput")
    wo_d = nc.dram_tensor("Wo", [DIM, DIM], BF16, kind="ExternalInput")
    mask_d = nc.dram_tensor("mask", [P, 2048], BF16, kind="ExternalInput")
    out_d = nc.dram_tensor("out", [DIM, OWN], BF16, kind="ExternalOutput")

    lsegs = [(0, 512), (512, 512), (1024, 256)]   # local-token segments
    osegs = [(0, 512), (512, 512)]                # owned-token segments

    with tile.TileContext(nc) as tc:
        ps = tc.alloc_tile_pool(name="ps", bufs=4, space="PSUM")

        def psum(shape):
            return ps.tile(shape, F32, tag="ps", name="pst", bufs=4)

        def psum_acc(shape):
            return ps.tile(shape, F32, tag="pacc", name="pacc", bufs=2)

        const_p = tc.alloc_tile_pool(name="const", bufs=1, side="left")
        mask_sb = const_p.tile([P, 2048], BF16, bufs=1)
        ones_f = const_p.tile([P, 1], F32, bufs=1)
        nc.vector.memset(ones_f[:], 1.0)
        ones_sb = const_p.tile([P, 1], F32R, bufs=1)
        nc.vector.tensor_copy(ones_sb[:], ones_f[:])
        # NOTE: onesr is unused by the compute, but removing it shifts the
        # Tile schedule and measurably regresses HW time (337us -> 396us).
        onesr_f = const_p.tile([1, DH], F32, bufs=1)
        nc.vector.memset(onesr_f[:], 1.0)
        onesr = const_p.tile([1, DH], F32R, bufs=1)
        nc.vector.tensor_copy(onesr[:], onesr_f[:])
        onesc_f = const_p.tile([1, P], F32, bufs=1)
        nc.vector.memset(onesc_f[:], 1.0)
        onesc = const_p.tile([1, P], F32R, bufs=1)
        nc.vector.tensor_copy(onesc[:], onesc_f[:])
        bg_sb = const_p.tile([HEADS, 1], F32, bufs=1)
        eps_sb = const_p.tile([1, 1], F32, bufs=1)
        nc.vector.memset(eps_sb[:], 1e-24)
        sgT = const_p.tile([HEADS, OWN], BF16, bufs=1)

        w_p = tc.alloc_tile_pool(name="w", bufs=24, side="right")
        xh_p = tc.alloc_tile_pool(name="xh", bufs=KK, side="right")
        x_p = tc.alloc_tile_pool(name="x", bufs=KK, side="right")
        x2_p = tc.alloc_tile_pool(name="x2", bufs=3, side="right")

        def wload(dram, kk, name):
            wt = w_p.tile([P, DIM], BF16, tag="w", name=name)
            nc.sync.dma_start(wt[:], dram[kk * P:(kk + 1) * P, :])
            return wt

        # seg-major x DMA: all 8 dim-chunks of token segment 0 land first,
        # so rs/xhat/K/Q for early segments start while the rest streams in.
        # Weight DMAs are interleaved between segments in first-use order.
        x_sb = [x_p.tile([P, SL], F32, tag="xT", name=f"x{kk}")
                for kk in range(KK)]
        for kk in range(KK):
            nc.sync.dma_start(x_sb[kk][:, 0:512], xT_d[kk * P:(kk + 1) * P,
                                                       0:512])
        wq_sb = [wload(wq_d, kk, f"wq{kk}") for kk in range(KK)]
        for kk in range(KK):
            nc.sync.dma_start(x_sb[kk][:, 512:1024],
                              xT_d[kk * P:(kk + 1) * P, 512:1024])
        wk_sb = [wload(wk_d, kk, f"wk{kk}") for kk in range(KK)]
        for kk in range(KK):
            nc.sync.dma_start(x_sb[kk][:, 1024:1280],
                              xT_d[kk * P:(kk + 1) * P, 1024:1280])
        nc.sync.dma_start(mask_sb[:], mask_d[:])
        nc.sync.dma_start(bg_sb[:], bg_d[:])

        # HAM warm-up: const-fed dummy matmuls keep the PE busy while the
        # xT DMA lands, so projections start at 2.4GHz instead of 1.2GHz.
        dmy_f = const_p.tile([1, 512], F32, bufs=1)
        nc.vector.memset(dmy_f[:], 1.0)
        dmy_r = const_p.tile([1, 512], F32R, bufs=1)
        nc.vector.tensor_copy(dmy_r[:], dmy_f[:])
        def emit_dummy(n):
            wps = psum([DH, 512])
            for j in range(n):
                nc.tensor.matmul(wps[:], onesr[:], dmy_r[:],
                                 start=(j == 0), stop=(j == n - 1))

        emit_dummy(16)

        # ---- norm: rs = 1/sqrt(sum_d x^2), one token segment at a time --
        rs_row = x2_p.tile([1, SL], F32, bufs=1)
        rsb = x2_p.tile([P, SL], F32, bufs=1)
        ssq_ps = [psum([1, w]) for _, w in lsegs]
        xh_sb = [xh_p.tile([P, SL], BF16, tag="xh", name=f"xh{kk}")
                 for kk in range(KK)]

        def emit_norm_seg(si):
            s0, w = lsegs[si]
            for kk in range(KK):
                x2 = x2_p.tile([P, 512], F32R, tag="x2",
                               name=f"x2_{si}_{kk}")
                nc.scalar.activation(x2[:, :w], x_sb[kk][:, s0:s0 + w],
                                     AF.Square)
                nc.tensor.matmul(ssq_ps[si][:], ones_sb[:], x2[:, :w],
                                 start=(kk == 0), stop=(kk == KK - 1))
            nrm = x2_p.tile([1, 512], F32, tag="nrm", name=f"nrm{si}")
            nc.scalar.activation(nrm[:1, :w], ssq_ps[si][:], AF.Sqrt,
                                 bias=eps_sb[:])
            nc.vector.reciprocal_approx_fast(rs_row[:, s0:s0 + w],
                                             nrm[:1, :w])
            nc.gpsimd.partition_broadcast(rsb[:, s0:s0 + w],
                                          rs_row[:, s0:s0 + w])
            for kk in range(KK):
                nc.vector.tensor_mul(xh_sb[kk][:, s0:s0 + w],
                                     x_sb[kk][:, s0:s0 + w],
                                     rsb[:, s0:s0 + w])
        # ---- projections ----------------------------------------------
        q_p = tc.alloc_tile_pool(name="q", bufs=FT, side="left")
        k_p = tc.alloc_tile_pool(name="k", bufs=FT, side="left")
        v_p = tc.alloc_tile_pool(name="v", bufs=TCH, side="left")

        # Q/K passes are emitted per feature-tile: ft 0-1 up front, ft 2-7
        # interleaved into the attention head loop as PE filler so the
        # attention phase (ACT/DVE-bound) overlaps the projections.
        qT = [q_p.tile([P, OWN], BF16, tag="qT", name=f"qT{ft}")
              for ft in range(FT)]
        kT = [k_p.tile([P, SL], BF16, tag="kT", name=f"kT{ft}")
              for ft in range(FT)]

        # filler evacuations alternate between ACT and DVE so neither
        # strict-FIFO queue becomes the attention-phase bottleneck
        evac_flip = [False]

        def evac(dst, acc):
            if evac_flip[0]:
                nc.vector.tensor_copy(dst, acc)
            else:
                nc.scalar.copy(dst, acc)
            evac_flip[0] = not evac_flip[0]

        def emit_qseg(ft, si):
            s0, w = osegs[si]
            acc = psum([P, w])
            for kk in range(KK):
                nc.tensor.matmul(
                    acc[:], wq_sb[kk][:, ft * P:(ft + 1) * P],
                    xh_sb[kk][:, HALO + s0:HALO + s0 + w],
                    start=(kk == 0), stop=(kk == KK - 1))
            evac(qT[ft][:, s0:s0 + w], acc[:])

        def emit_kseg(ft, si):
            s0, w = lsegs[si]
            acc = psum([P, w])
            for kk in range(KK):
                nc.tensor.matmul(
                    acc[:], wk_sb[kk][:, ft * P:(ft + 1) * P],
                    xh_sb[kk][:, s0:s0 + w],
                    start=(kk == 0), stop=(kk == KK - 1))
            evac(kT[ft][:, s0:s0 + w], acc[:])

        def emit_q(ft):
            for si in range(len(osegs)):
                emit_qseg(ft, si)

        def emit_k(ft):
            for si in range(len(lsegs)):
                emit_kseg(ft, si)

        # interleaved norm/projection emission: each token segment's
        # rs/xhat unlocks its K (and, one segment later, Q) columns while
        # later x segments are still streaming in; dummy matmuls bridge
        # the remaining DMA/DVE waits
        emit_norm_seg(0)
        emit_dummy(8)
        emit_kseg(0, 0)
        emit_kseg(1, 0)
        emit_norm_seg(1)
        emit_dummy(8)
        emit_qseg(0, 0)
        emit_qseg(1, 0)
        emit_kseg(0, 1)
        emit_kseg(1, 1)
        emit_norm_seg(2)
        emit_dummy(8)
        emit_qseg(0, 1)
        emit_qseg(1, 1)
        emit_kseg(0, 2)
        emit_kseg(1, 2)
        x2_p.release()
        x_p.release()

        # gates -> sigmoid(x @ Wg + bg), head-major [16, 1024] (before V so
        # attention can start right after the fh=0 half of the V pass)
        wg_p = tc.alloc_tile_pool(name="wg", bufs=KK, side="right")
        wg_sb = []
        for kk in range(KK):
            wgt = wg_p.tile([P, HEADS], BF16, tag="wg", name=f"wgk{kk}")
            nc.sync.dma_start(wgt[:], wg_d[kk * P:(kk + 1) * P, :])
            wg_sb.append(wgt)
        for s0, w in osegs:
            acc = psum([HEADS, w])
            for kk in range(KK):
                nc.tensor.matmul(acc[:], wg_sb[kk][:],
                                 xh_sb[kk][:, HALO + s0:HALO + s0 + w],
                                 start=(kk == 0), stop=(kk == KK - 1))
            nc.scalar.activation(sgT[:, s0:s0 + w], acc[:], AF.Sigmoid,
                                 bias=bg_sb[:])

        # V pass -> token-major with interleaved ones columns.  Only the
        # fh=0 half (heads 0-7) is emitted here; the fh=1 half is
        # interleaved into the first 8 attention heads to keep the PE busy
        # (and HAM un-throttled) while attention runs on ACT/DVE.
        wv_sb = [wload(wv_d, kk, f"wv{kk}") for kk in range(KK)]
        v_sb = []
        for g in range(TCH):
            vt = v_p.tile([P, HEADS * (DH + 1)], BF16, tag="v", name=f"v{g}")
            v3 = vt.rearrange("p (h e) -> p h e", e=DH + 1)
            nc.vector.memset(v3[:, :, DH:DH + 1], 1.0)
            acc = psum([P, 512])
            for kk in range(KK):
                nc.tensor.matmul(
                    acc[:], xh_sb[kk][:, g * P:(g + 1) * P],
                    wv_sb[kk][:, 0:512],
                    start=(kk == 0), stop=(kk == KK - 1))
            nc.vector.tensor_copy(v3[:, 0:8, 0:DH], acc[:])
            v_sb.append(v3)

        def emit_vfh1(g):
            acc = psum([P, 512])
            for kk in range(KK):
                nc.tensor.matmul(
                    acc[:], xh_sb[kk][:, g * P:(g + 1) * P],
                    wv_sb[kk][:, 512:1024],
                    start=(kk == 0), stop=(kk == KK - 1))
            a3 = acc.rearrange("p (h e) -> p h e", e=DH)
            nc.scalar.copy(v_sb[g][:, 8:16, 0:DH], a3[:])

        # ---- attention --------------------------------------------------
        ag_p = tc.alloc_tile_pool(name="ag", bufs=FT, side="right")
        wo_p = tc.alloc_tile_pool(name="wo", bufs=KK, side="right")
        e_p = tc.alloc_tile_pool(name="e", bufs=20, side="right")
        av_p = tc.alloc_tile_pool(name="av", bufs=2, side="right")
        wo_sb = []
        for t in range(KK):
            wot = wo_p.tile([P, DIM], BF16, tag="wo", name=f"wo{t}")
            nc.sync.dma_start(wot[:], wo_d[t * P:(t + 1) * P, :])
            wo_sb.append(wot)
        agT = [ag_p.tile([P, OWN], BF16, tag="agT", name=f"agT{ft}")
               for ft in range(FT)]
        # persistent double-buffered eT tiles: the zero pads outside each
        # g's statically-valid range are memset ONCE here instead of per
        # head (exp/mask only ever write the valid range)
        # chunk pairs (0,1) and (8,9) share one [P,512] tile: one score
        # PSUM bank, one exp, one mask multiply (their mask sections are
        # adjacent).  The full-width mask multiply also writes every
        # invalid column, so those tiles need no pad memsets.
        e_bufs = []
        for d in range(2):
            row = [None] * TCH
            e01 = e_p.tile([P, 512], BF16, tag=f"e01d{d}", bufs=1,
                           name=f"e01d{d}")
            e89 = e_p.tile([P, 512], BF16, tag=f"e89d{d}", bufs=1,
                           name=f"e89d{d}")
            row[0] = e01[:, 0:256]
            row[1] = e01[:, 256:512]
            row[8] = e89[:, 0:256]
            row[9] = e89[:, 256:512]
            for g in range(2, 8):
                qs, w = _G_SPAN[g]
                v0, vw = _G_VALID[g]
                e = e_p.tile([P, w], BF16, tag=f"e{g}d{d}", bufs=1,
                             name=f"e{g}d{d}")
                if v0 > 0:
                    nc.vector.memset(e[:, 0:v0], 0.0)
                if v0 + vw < w:
                    nc.vector.memset(e[:, v0 + vw:w], 0.0)
                row[g] = e
            e_bufs.append((row, e01, e89))

        def emit_scores(ft, h2):
            h = 2 * ft + h2
            hp = h2 * DH
            # gate row for head h staged at partition 0 (HW
            # partition_broadcast always reads physical partition 0);
            # DMA is exempt from the engine partition-alignment rules
            sg0 = av_p.tile([1, OWN], BF16, tag="sg0", name=f"sg0_{h}",
                            bufs=3)
            nc.sync.dma_start(sg0[:], sgT[h:h + 1, :])
            eT, e01, e89 = e_bufs[h % 2]

            def pair(ga, etile, mc):
                # two 256-wide key chunks share one PSUM bank -> one exp
                # and one mask multiply over [P, 512]
                sc = psum([P, 512])
                for j in range(2):
                    g = ga + j
                    qs, w = _G_SPAN[g]
                    nc.tensor.matmul(
                        sc[:, 256 * j:256 * j + 256],
                        kT[ft][hp:hp + DH, g * P:(g + 1) * P],
                        qT[ft][hp:hp + DH, qs:qs + 256],
                        start=True, stop=True, skip_group_check=(j == 1))
                nc.scalar.activation(etile[:], sc[:], AF.Exp)
                nc.vector.tensor_mul(etile[:], etile[:],
                                     mask_sb[:, mc:mc + 512])

            for g in (2, 3, 6, 7, 4, 5):
                qs, w = _G_SPAN[g]
                v0, vw = _G_VALID[g]
                sc = psum([P, vw])
                nc.tensor.matmul(
                    sc[:], kT[ft][hp:hp + DH, g * P:(g + 1) * P],
                    qT[ft][hp:hp + DH, qs + v0:qs + v0 + vw],
                    start=True, stop=True)
                e = eT[g]
                nc.scalar.activation(e[:, v0:v0 + vw], sc[:], AF.Exp)
                mc = _G_MASK[g]
                nc.vector.tensor_mul(e[:, v0:v0 + vw], e[:, v0:v0 + vw],
                                     mask_sb[:, mc + v0:mc + v0 + vw])
            pair(0, e01, 1024)
            pair(8, e89, 1536)
            return sg0, eT

        def emit_av(ft, h2, sg0, eT):
            h = 2 * ft + h2
            hp = h2 * DH
            # whole head in one [65, 1024] two-bank PSUM accumulation;
            # blockpair i lands in bank i (cols 512i..512i+512).
            acc = psum_acc([DH + 1, OWN])
            for i in range(2):
                # full-width matmuls first so start=True overwrites the
                # whole bank before partial-width accumulates land.
                base = 4 * i
                c0 = 2 * i * WIN
                parts = [(base + 2, 0, 0, 512), (base + 3, 0, 0, 512),
                         (base + 0, 2 * i * WIN - _G_SPAN[base][0], 0, WIN),
                         (base + 1, 2 * i * WIN - _G_SPAN[base + 1][0],
                          0, WIN),
                         (base + 4,
                          (2 * i + 1) * WIN - _G_SPAN[base + 4][0],
                          WIN, WIN),
                         (base + 5,
                          (2 * i + 1) * WIN - _G_SPAN[base + 5][0],
                          WIN, WIN)]
                for j, (g, sect, p0, pw) in enumerate(parts):
                    nc.tensor.matmul(
                        acc[:, c0 + p0:c0 + p0 + pw], v_sb[g][:, h, :],
                        eT[g][:, sect:sect + pw],
                        start=(j == 0), stop=(j == len(parts) - 1),
                        skip_group_check=True)
            # scale = sigmoid(gate)/denominator; the denominator row is
            # evacuated by the scalar engine (gpsimd can't read PSUM),
            # reciprocal + gate-multiply on DVE, then one broadcast and a
            # single [64, 1024] multiply.
            srow = av_p.tile([1, OWN], F32, tag="srow", name="sr", bufs=1)
            nc.scalar.copy(srow[:], acc[DH:DH + 1, :])
            sinv = av_p.tile([1, OWN], F32, tag="sinv", name="si", bufs=1)
            nc.vector.reciprocal_approx_fast(sinv[:], srow[:])
            crow = av_p.tile([1, OWN], BF16, tag="crow", name="cr")
            nc.vector.tensor_mul(crow[:], sinv[:], sg0[:])
            cb = av_p.tile([DH, OWN], BF16, tag="cb", name="cb")
            nc.gpsimd.partition_broadcast(cb[:], crow[:])
            nc.vector.tensor_mul(agT[ft][hp:hp + DH, :], acc[0:DH, :], cb[:])


        # PE filler work interleaved into the head loop at segment
        # granularity: remaining Q/K feature tiles and the fh=1 half of the
        # V pass, spread over ALL 16 heads (respecting each unit's deadline:
        # q/k of ft before head 2*ft, v1 before AV(8)).  This keeps the PE
        # near-continuously busy so HAM never re-throttles the clock.
        # Light heads get const-fed dummy matmuls as ballast.
        fill = {
            0: [("v1", 0), ("v1", 1)],
            1: [("v1", 2), ("q", (2, 0))],
            2: [("q", (2, 1)), ("k", (2, 0)), ("k", (2, 1))],
            3: [("k", (2, 2)), ("v1", 3)],
            4: [("q", (3, 0)), ("q", (3, 1)), ("k", (3, 0))],
            5: [("k", (3, 1)), ("k", (3, 2)), ("v1", 4)],
            6: [("q", (4, 0)), ("q", (4, 1)), ("k", (4, 0))],
            7: [("k", (4, 1)), ("k", (4, 2)), ("v1", 5)],
            8: [("v1", 6), ("v1", 7), ("q", (5, 0))],
            9: [("v1", 8), ("v1", 9), ("q", (5, 1))],
            10: [("k", (5, 0)), ("k", (5, 1)), ("k", (5, 2))],
            11: [("q", (6, 0)), ("q", (6, 1))],
            12: [("k", (6, 0)), ("k", (6, 1)), ("k", (6, 2))],
            13: [("q", (7, 0)), ("q", (7, 1))],
            14: [("k", (7, 0)), ("k", (7, 1)), ("k", (7, 2))],
            15: [("d", 8)],
        }

        def run_fill(kind, idx):
            if kind == "q":
                emit_qseg(*idx)
            elif kind == "k":
                emit_kseg(*idx)
            elif kind == "v1":
                emit_vfh1(idx)
            elif kind == "d":
                emit_dummy(idx)

        from collections import deque
        pend = deque()
        for ft in range(FT):
            for h2 in range(2):
                h = 2 * ft + h2
                for kind, idx in fill.get(h, ()):
                    run_fill(kind, idx)
                pend.append((ft, h2, *emit_scores(ft, h2)))
                if len(pend) > 1:
                    emit_av(*pend.popleft())
        while pend:
            emit_av(*pend.popleft())

        # keep the PE busy during the last heads' epilogue drain so the
        # output projection starts un-throttled
        emit_dummy(12)
        emit_dummy(12)

        # ---- output projection -----------------------------------------
        av_p.release()
        e_p.release()
        v_p.release()
        k_p.release()
        q_p.release()
        y_p = tc.alloc_tile_pool(name="y", bufs=3, side="right")
        for dt in range(KK):
            yt = y_p.tile([P, OWN], BF16, tag="yt", name=f"yt{dt}")
            for s0, w in osegs:
                acc = psum([P, w])
                for t in range(KK):
                    nc.tensor.matmul(acc[:], wo_sb[t][:, dt * P:(dt + 1) * P],
                                     agT[t][:, s0:s0 + w],
                                     start=(t == 0), stop=(t == KK - 1))
                nc.scalar.copy(yt[:, s0:s0 + w], acc[:])
            nc.sync.dma_start(out_d[dt * P:(dt + 1) * P, :], yt[:])
        y_p.release()
        wo_p.release()
        ag_p.release()
        wg_p.release()
        xh_p.release()
        w_p.release()
        const_p.release()
        ps.release()

    nc.compile()
    return nc


def make_in_maps(x, gamma, W_qkv, W_gates, b_gates, W_out):
    b, S, dim = x.shape
    assert (b, S, dim) == (2, 4096, DIM)
    g32 = (gamma * (dim ** 0.5)).astype(np.float32)
    wqkv = W_qkv * g32[:, None]
    wq = (wqkv[:, :DIM] * (DH ** -0.5)).astype(ml_dtypes.bfloat16)
    wk = np.asarray(wqkv[:, DIM:2 * DIM], np.float32).astype(ml_dtypes.bfloat16)
    wv = np.asarray(wqkv[:, 2 * DIM:3 * DIM],
                    np.float32).astype(ml_dtypes.bfloat16)
    wg = (W_gates * g32[:, None]).astype(ml_dtypes.bfloat16)
    wo = np.asarray(W_out, np.float32).astype(ml_dtypes.bfloat16)
    bg = np.ascontiguousarray(b_gates, dtype=np.float32)
    m_first = _masks(True)
    m_rest = _masks(False)

    in_maps = []
    for c in range(NCORES):
        bb, seg = c // 4, c % 4
        own = x[bb, seg * OWN:(seg + 1) * OWN]
        halo = x[bb, seg * OWN - HALO: seg * OWN] if seg else x[bb, :HALO]
        xT = np.ascontiguousarray(
            np.concatenate([halo, own], axis=0).T, dtype=np.float32)
        in_maps.append({
            "xT": xT, "Wq": wq, "Wk": wk, "Wv": wv, "Wg": wg, "bg": bg,
            "Wo": wo, "mask": m_first if seg == 0 else m_rest,
        })
    return in_maps


_NC_CACHE = []


def kernel(x, gamma, W_qkv, W_gates, b_gates, W_out):
    x = np.asarray(x, dtype=np.float32)
    in_maps = make_in_maps(
        x, np.asarray(gamma, np.float32), np.asarray(W_qkv, np.float32),
        np.asarray(W_gates, np.float32), np.asarray(b_gates, np.float32),
        np.asarray(W_out, np.float32))
    if not _NC_CACHE:
        _NC_CACHE.append(build())
    nc = _NC_CACHE[0]
    res = run_bass_kernel_spmd(nc, in_maps, core_ids=list(range(NCORES)))
    y = np.empty((2, 4096, DIM), dtype=np.float32)
    for c in range(NCORES):
        bb, seg = c // 4, c % 4
        y[bb, seg * OWN:(seg + 1) * OWN] = res.results[c]["out"].T
    return y

